# revision 18
# baseline (speedup 1.0000x reference)
"""Trainium2 Bass kernel for nn_Distance_Module (retrieval_knn).

Math: out[i,j] = (dmax[i]-mn)/(mx-mn) off-diagonal, (dmin[i]-mn)/(mx-mn)
on the diagonal, where per sample i:
  s[t,f] = <text[i,t]/|..|, video[i,f]/|..|>, dmin[i] = 1-max s, dmax[i] = 1-min s,
mn = min_i dmin[i], mx = max_i dmax[i].

Device kernel (SPMD x8, batch-sharded, no collectives): per core 64 samples.
Host uploads each core's shard pre-cast to bf16 and pre-transposed to
D-major ([512, rows]); the device reads it with large contiguous DMA
descriptors (memory-roofline bound), computes row sq-norms with one bf16
squares pass + per-sample ones-matmuls (PSUM column accumulation), raw
similarity matrices with per-sample bf16 matmuls, folds the 1/(|x||y|)
normalization in as a rank-1 outer-product tile, and reduces min/max per
sample. Host: gather 8x[64,2] vectors, global min/max, build [512,512].
"""

from contextlib import ExitStack

import numpy as np

import concourse.bass as bass
import concourse.tile as tile
from concourse import masks, mybir
from concourse.bass_utils import run_bass_kernel_spmd
from concourse.vector_clock import ScopedClock

# The walrus in this toolchain only allows ONE sync-wait per instruction;
# TileContext's tail drain attaches one wait per outstanding semaphore and
# fails codegen. Split them across consecutive drains / NoOps.
_MAX_CTRL_WAITS = 1


def _split_drain_and_barrier(self, tick_clock, wait_clock):
    nc = self.nc
    drain_inst = nc.sync.drain()
    wait_clock.add_sem_waits(
        drain_inst.ins, ScopedClock({None: tick_clock.global_clock})
    )
    si = drain_inst.ins.sync_info
    waits = list(si.on_wait or []) if si else []
    if len(waits) > _MAX_CTRL_WAITS:
        si.on_wait = waits[:_MAX_CTRL_WAITS]
        for i in range(_MAX_CTRL_WAITS, len(waits), _MAX_CTRL_WAITS):
            extra = nc.sync.drain()
            esi = extra.ins.sync_info
            chunk = waits[i : i + _MAX_CTRL_WAITS]
            if esi is None:
                extra.ins.sync_info = mybir.SyncInfo(on_wait=chunk, on_update=[])
            else:
                esi.on_wait = chunk
    nc.all_engine_barrier()
    assert self.sems is not None
    popped = nc._tile_sem_poison_stack.pop()
    assert popped is self._sem_poison
    nc.clear_and_free_semaphores(list(self.sems.allocated().values()))
    # No trailing all-engine barrier: the clears are the last instructions,
    # NEFF completion already waits for every engine queue to retire, and
    # this is the outermost (only) TileContext so nothing follows.


tile.TileContext._drain_and_barrier = _split_drain_and_barrier


def _split_sync_waits(nc, max_waits=_MAX_CTRL_WAITS):
    """Hoist extra sync-waits onto same-engine NoOps inserted just before
    the offending instruction."""
    f = nc.m.functions[0]
    for blk in f.blocks:
        out = []
        for inst in blk.instructions:
            si = getattr(inst, "sync_info", None)
            waits = list(si.on_wait) if (si and si.on_wait) else []
            if len(waits) > max_waits:
                for i in range(0, len(waits) - max_waits, max_waits):
                    nop = mybir.InstNoOp(
                        name=nc.get_next_instruction_name(), ins=[], outs=[]
                    )
                    nop.engine = inst.engine
                    nop.sync_info = mybir.SyncInfo(
                        on_wait=waits[i : i + max_waits], on_update=[]
                    )
                    nc.register_instruction(nop)
                    out.append(nop)
                si.on_wait = waits[len(waits) - max_waits :]
            out.append(inst)
        blk.instructions[:] = out


B, T, F, D = 512, 77, 12, 512
NCORES = 8
BS = B // NCORES  # 64 samples per core
XROWS = BS * T  # 4928
YROWS = BS * F  # 768
NCH = D // 128  # 4 contraction chunks
GRP = 16  # samples per pipeline group
NG = BS // GRP  # 4 groups
XGW = GRP * T  # 1232 x-rows per group

FP32 = mybir.dt.float32
BF16 = mybir.dt.bfloat16
ALU = mybir.AluOpType
AX = mybir.AxisListType
ACTF = mybir.ActivationFunctionType

# Engine split for elementwise passes: "d"=DVE tensor_tensor,
# "a"=ACT Square activation, "p"=Pool tensor_tensor.
SQY_PAT = "dada"  # y squares, per chunk (768 cols each)
YN_PAT = "dddp"  # ynorm scale-mult, per chunk


def _sq_op(nc, eng, out_ap, in_ap):
    if eng == "a":
        nc.scalar.activation(out_ap, in_ap, ACTF.Square, 0.0, 1.0)
    elif eng == "d":
        nc.vector.tensor_tensor(out_ap, in_ap, in_ap, ALU.mult)
    else:
        nc.gpsimd.tensor_tensor(out_ap, in_ap, in_ap, ALU.mult)


def _build_body(ctx: ExitStack, tc: "tile.TileContext", textT, videoT, dout):
    nc = tc.nc

    const_pool = ctx.enter_context(tc.tile_pool(name="const", bufs=1))
    ones = const_pool.tile([128, 128], BF16)
    nc.gpsimd.memset(ones[:], 1.0)

    # Persistent D-major bf16 shards: [128, c, rows] (partition = d%128).
    big_pool = ctx.enter_context(tc.tile_pool(name="big", bufs=1))
    Xbf = big_pool.tile([128, NCH * XROWS], BF16)
    Ybf = big_pool.tile([128, NCH * YROWS], BF16)
    sqY = big_pool.tile([128, NCH * YROWS], BF16)
    Ynm = big_pool.tile([128, NCH * YROWS], BF16)  # normalized y
    invY = big_pool.tile([128, YROWS], FP32)
    rnyB = big_pool.tile([128, YROWS], BF16)  # 1/|y| bcast down partitions
    dvAll = big_pool.tile([1, 2 * BS], FP32)  # smax[0:BS], smin[BS:2BS]

    sq_pool = ctx.enter_context(tc.tile_pool(name="sq", bufs=3))
    rnx_pool = ctx.enter_context(tc.tile_pool(name="rnx", bufs=2))
    h_pool = ctx.enter_context(tc.tile_pool(name="h", bufs=2))

    psA_pool = ctx.enter_context(tc.tile_pool(name="psA", bufs=1, space="PSUM"))
    npsX = psA_pool.tile([T, BS], FP32)
    ry_pool = ctx.enter_context(tc.tile_pool(name="ry", bufs=2, space="PSUM"))
    g_pool = ctx.enter_context(tc.tile_pool(name="g", bufs=4, space="PSUM"))

    Xv = Xbf[:].rearrange("p (c w) -> p c w", c=NCH)
    Yv = Ybf[:].rearrange("p (c w) -> p c w", c=NCH)
    sqYv = sqY[:].rearrange("p (c w) -> p c w", c=NCH)
    Ynv = Ynm[:].rearrange("p (c w) -> p c w", c=NCH)
    xsrc = textT.ap().rearrange("(c p) w -> p c w", p=128)
    ysrc = videoT.ap().rearrange("(c p) w -> p c w", p=128)

    # --- all input DMAs issued up front; transfers pipeline back-to-back ---
    HGW = XGW // 2  # half-group: 8 samples = 616 rows per DMA / sq block
    nc.sync.dma_start(Yv, ysrc)
    for g in range(NG):
        for hh in range(2):
            w0 = g * XGW + hh * HGW
            nc.sync.dma_start(Xv[:, :, w0 : w0 + HGW], xsrc[:, :, w0 : w0 + HGW])

    # --- Y: squares, dup norm matmuls, rsqrt, normalize ---
    for c in range(NCH):
        _sq_op(nc, SQY_PAT[c], sqYv[:, c, :], Yv[:, c, :])
    HW = YROWS // 2  # psum bank is 2KB; [128, 384] fp32 halves
    for h in range(2):
        ry = ry_pool.tile([128, HW], FP32, tag="ry", name=f"ry{h}")
        for c in range(NCH):
            nc.tensor.matmul(
                ry[:, :],
                ones[:, :128],
                sqYv[:, c, h * HW : (h + 1) * HW],
                start=(c == 0),
                stop=(c == NCH - 1),
            )
        nc.vector.reciprocal(invY[:, h * HW : (h + 1) * HW], ry[:, :])
    nc.scalar.sqrt(rnyB[:, :], invY[:, :])
    for c in range(NCH):
        eng = YN_PAT[c]
        if eng == "d":
            nc.vector.tensor_tensor(Ynv[:, c, :], Yv[:, c, :], rnyB[:, :], ALU.mult)
        else:
            nc.gpsimd.tensor_tensor(Ynv[:, c, :], Yv[:, c, :], rnyB[:, :], ALU.mult)

    # --- X groups, software-pipelined: squares+norms for g, then finish g-1
    def emit_half(g, hh, G):
        """raw similarity matmuls + squares + sqnorm matmuls for 8 samples"""
        w0 = g * XGW + hh * HGW
        w1 = w0 + HGW
        for j in range(GRP // 2):
            b = g * GRP + hh * (GRP // 2) + j
            jj = hh * (GRP // 2) + j
            for c in range(NCH):
                nc.tensor.matmul(
                    G[:, jj * F : (jj + 1) * F],
                    Xv[:, c, b * T : (b + 1) * T],
                    Ynv[:, c, b * F : (b + 1) * F],
                    start=(c == 0),
                    stop=(c == NCH - 1),
                )
        sq = sq_pool.tile([128, NCH * HGW], BF16, tag="sq", name=f"sq{g}_{hh}")
        sqv = sq[:].rearrange("p (c w) -> p c w", c=NCH)
        # balanced split: DVE c0,c1; ACT c2; Pool c3
        _sq_op(nc, "d", sqv[:, 0, :], Xv[:, 0, w0:w1])
        _sq_op(nc, "d", sqv[:, 1, :], Xv[:, 1, w0:w1])
        _sq_op(nc, "a", sqv[:, 2, :], Xv[:, 2, w0:w1])
        _sq_op(nc, "p", sqv[:, 3, :], Xv[:, 3, w0:w1])
        for j in range(GRP // 2):
            b = g * GRP + hh * (GRP // 2) + j
            for c in range(NCH):
                nc.tensor.matmul(
                    npsX[:, b : b + 1],
                    sqv[:, c, j * T : (j + 1) * T],
                    ones[:, :1],
                    start=(c == 0),
                    stop=(c == NCH - 1),
                )

    def emit_front(g):
        G = g_pool.tile([T, GRP * F], FP32, tag="g", name=f"g{g}")
        emit_half(g, 0, G)
        emit_half(g, 1, G)
        return G

    def emit_finish(g, G):
        """rnx, f-reduce of G, scale, partition-reduce into dvAll"""
        s0 = g * GRP
        invx = rnx_pool.tile([T, GRP], FP32, tag="invx", name=f"invx{g}")
        nc.vector.reciprocal(invx[:, :], npsX[:, s0 : s0 + GRP])
        rnx = rnx_pool.tile([T, GRP], FP32, tag="rnx", name=f"rnx{g}")
        nc.scalar.sqrt(rnx[:, :], invx[:, :])
        Gv = G[:].rearrange("p (j f) -> p j f", f=F)
        gmx = h_pool.tile([T, GRP], FP32, tag="gmx", name=f"gmx{g}")
        gmn = h_pool.tile([T, GRP], FP32, tag="gmn", name=f"gmn{g}")
        nc.vector.tensor_reduce(gmx[:, :], Gv, axis=AX.X, op=ALU.max)
        # negate: gmn = -min_f(G); cross-lane reduce only supports max
        nc.vector.tensor_reduce(gmn[:, :], Gv, axis=AX.X, op=ALU.min, negate=True)
        hmx = h_pool.tile([T, GRP], FP32, tag="hmx", name=f"hmx{g}")
        hmn = h_pool.tile([T, GRP], FP32, tag="hmn", name=f"hmn{g}")
        nc.gpsimd.tensor_tensor(hmx[:, :], gmx[:, :], rnx[:, :], ALU.mult)
        nc.gpsimd.tensor_tensor(hmn[:, :], gmn[:, :], rnx[:, :], ALU.mult)
        nc.gpsimd.tensor_reduce(
            dvAll[:, s0 : s0 + GRP], hmx[:, :], axis=AX.C, op=ALU.max
        )
        nc.gpsimd.tensor_reduce(
            dvAll[:, BS + s0 : BS + s0 + GRP], hmn[:, :], axis=AX.C, op=ALU.max
        )

    prev = None
    for g in range(NG):
        G = emit_front(g)
        if prev is not None:
            emit_finish(*prev)
        prev = (g, G)
    emit_finish(*prev)

    nc.sync.dma_start(dout.ap(), dvAll[:, :])


def build():
    nc = bass.Bass("TRN2", target_bir_lowering=False, debug=False)
    textT = nc.dram_tensor("textT", [D, XROWS], BF16, kind="ExternalInput")
    videoT = nc.dram_tensor("videoT", [D, YROWS], BF16, kind="ExternalInput")
    dout = nc.dram_tensor("dout", [1, 2 * BS], FP32, kind="ExternalOutput")
    with tile.TileContext(nc) as tc:
        with ExitStack() as ctx:
            _build_body(ctx, tc, textT, videoT, dout)
    _split_sync_waits(nc)
    return nc


_nc_cache = None


def _get_nc():
    global _nc_cache
    if _nc_cache is None:
        _nc_cache = build()
    return _nc_cache


def _bf16():
    import ml_dtypes

    return np.dtype(ml_dtypes.bfloat16)


def prep_core_inputs(text: np.ndarray, video: np.ndarray, core: int) -> dict:
    """bf16-cast + D-major transpose of one core's shard (host-side prep)."""
    bf = _bf16()
    xs = text[core * BS : (core + 1) * BS].astype(bf).reshape(XROWS, D).T
    ys = video[core * BS : (core + 1) * BS].astype(bf).reshape(YROWS, D).T
    return {
        "textT": np.ascontiguousarray(xs),
        "videoT": np.ascontiguousarray(ys),
    }


def run_device(text: np.ndarray, video: np.ndarray, trace: bool = False):
    """Run the SPMD kernel on 8 cores; returns (smax[B], smin[B], results)."""
    nc = _get_nc()
    in_maps = [prep_core_inputs(text, video, i) for i in range(NCORES)]
    res = run_bass_kernel_spmd(nc, in_maps, list(range(NCORES)), trace=trace)
    douts = [np.asarray(res.results[i]["dout"]) for i in range(NCORES)]
    smax = np.concatenate([d[0, :BS] for d in douts])
    smin = np.concatenate([-d[0, BS:] for d in douts])
    return smax, smin, res


def kernel(Prob_text: np.ndarray, Prob_video: np.ndarray) -> np.ndarray:
    text = np.ascontiguousarray(np.asarray(Prob_text, dtype=np.float32))
    video = np.ascontiguousarray(np.asarray(Prob_video, dtype=np.float32))
    smax, smin, _ = run_device(text, video)
    dmin = 1.0 - smax.astype(np.float64)
    dmax = 1.0 - smin.astype(np.float64)
    mn = dmin.min()
    mx = dmax.max()
    dis = np.broadcast_to(dmax[:, None], (B, B)).copy()
    np.fill_diagonal(dis, dmin)
    return ((dis - mn) / (mx - mn)).astype(np.float32)


# revision 21
# speedup vs baseline: 1.1978x; 1.1978x over previous
"""Trainium2 Bass kernel for nn_Distance_Module (retrieval_knn).

Math: out[i,j] = (dmax[i]-mn)/(mx-mn) off-diagonal, (dmin[i]-mn)/(mx-mn)
on the diagonal, where per sample i:
  s[t,f] = <text[i,t]/|..|, video[i,f]/|..|>, dmin[i] = 1-max s, dmax[i] = 1-min s,
mn = min_i dmin[i], mx = max_i dmax[i].

Device kernel (SPMD x8, batch-sharded, no collectives): per core 64 samples.
Host uploads each core's shard pre-cast to bf16 and pre-transposed to
D-major ([512, rows]); the device reads it with large contiguous DMA
descriptors (memory-roofline bound), computes row sq-norms with one bf16
squares pass + per-sample ones-matmuls (PSUM column accumulation), raw
similarity matrices with per-sample bf16 matmuls, folds the 1/(|x||y|)
normalization in as a rank-1 outer-product tile, and reduces min/max per
sample. Host: gather 8x[64,2] vectors, global min/max, build [512,512].
"""

from contextlib import ExitStack

import numpy as np

import concourse.bass as bass
import concourse.tile as tile
from concourse import masks, mybir
from concourse.bass_utils import run_bass_kernel_spmd
from concourse.vector_clock import ScopedClock

# The walrus in this toolchain only allows ONE sync-wait per instruction;
# TileContext's tail drain attaches one wait per outstanding semaphore and
# fails codegen. Split them across consecutive drains / NoOps.
_MAX_CTRL_WAITS = 1


def _split_drain_and_barrier(self, tick_clock, wait_clock):
    nc = self.nc
    drain_inst = nc.sync.drain()
    wait_clock.add_sem_waits(
        drain_inst.ins, ScopedClock({None: tick_clock.global_clock})
    )
    si = drain_inst.ins.sync_info
    waits = list(si.on_wait or []) if si else []
    if len(waits) > _MAX_CTRL_WAITS:
        si.on_wait = waits[:_MAX_CTRL_WAITS]
        for i in range(_MAX_CTRL_WAITS, len(waits), _MAX_CTRL_WAITS):
            extra = nc.sync.drain()
            esi = extra.ins.sync_info
            chunk = waits[i : i + _MAX_CTRL_WAITS]
            if esi is None:
                extra.ins.sync_info = mybir.SyncInfo(on_wait=chunk, on_update=[])
            else:
                esi.on_wait = chunk
    nc.all_engine_barrier()
    assert self.sems is not None
    popped = nc._tile_sem_poison_stack.pop()
    assert popped is self._sem_poison
    nc.clear_and_free_semaphores(list(self.sems.allocated().values()))
    # No trailing all-engine barrier: the clears are the last instructions,
    # NEFF completion already waits for every engine queue to retire, and
    # this is the outermost (only) TileContext so nothing follows.


tile.TileContext._drain_and_barrier = _split_drain_and_barrier


def _split_sync_waits(nc, max_waits=_MAX_CTRL_WAITS):
    """Hoist extra sync-waits onto same-engine NoOps inserted just before
    the offending instruction."""
    f = nc.m.functions[0]
    for blk in f.blocks:
        out = []
        for inst in blk.instructions:
            si = getattr(inst, "sync_info", None)
            waits = list(si.on_wait) if (si and si.on_wait) else []
            if len(waits) > max_waits:
                for i in range(0, len(waits) - max_waits, max_waits):
                    nop = mybir.InstNoOp(
                        name=nc.get_next_instruction_name(), ins=[], outs=[]
                    )
                    nop.engine = inst.engine
                    nop.sync_info = mybir.SyncInfo(
                        on_wait=waits[i : i + max_waits], on_update=[]
                    )
                    nc.register_instruction(nop)
                    out.append(nop)
                si.on_wait = waits[len(waits) - max_waits :]
            out.append(inst)
        blk.instructions[:] = out


B, T, F, D = 512, 77, 12, 512
NCORES = 8
BS = B // NCORES  # 64 samples per core
XROWS = BS * T  # 4928
YROWS = BS * F  # 768
NCH = D // 128  # 4 contraction chunks
GRP = 16  # samples per finish group
NG = BS // GRP  # 4 groups
XGW = GRP * T  # 1232 x-rows per group

FP32 = mybir.dt.float32
BF16 = mybir.dt.bfloat16
FP8 = mybir.dt.float8e4
ALU = mybir.AluOpType
AX = mybir.AxisListType
ACTF = mybir.ActivationFunctionType

# Input dtypes for similarity operands: BF16 or FP8 (e4m3).
X_DT = FP8
Y_DT = BF16

# Engine split tables: "d"=DVE tensor_tensor, "a"=ACT Square, "p"=Pool TT.
SQY_PAT = "dada"  # y squares, per chunk (768 cols each)
SQX_PATS = ["dapd", "adpa"]  # x squares per half-group chunk, cycled
G2_ENG = "d"  # must not be "p": gpsimd cannot read PSUM  # G*rny scale engine (p=Pool, d=DVE)
MULT_ENG = "p"  # hmx/hmn scale engine


def _sq_op(nc, eng, out_ap, in_ap):
    if eng == "a":
        nc.scalar.activation(out_ap, in_ap, ACTF.Square, 0.0, 1.0)
    elif eng == "d":
        nc.vector.tensor_tensor(out_ap, in_ap, in_ap, ALU.mult)
    else:
        nc.gpsimd.tensor_tensor(out_ap, in_ap, in_ap, ALU.mult)


def _build_body(ctx: ExitStack, tc: "tile.TileContext", textT, videoT, dout):
    nc = tc.nc

    const_pool = ctx.enter_context(tc.tile_pool(name="const", bufs=1))
    ones = const_pool.tile([128, 128], BF16)
    nc.gpsimd.memset(ones[:], 1.0)

    # Persistent D-major shards: [128, c, rows] (partition = d%128).
    big_pool = ctx.enter_context(tc.tile_pool(name="big", bufs=1))
    Xq = big_pool.tile([128, NCH * XROWS], X_DT)
    Yq = big_pool.tile([128, NCH * YROWS], Y_DT)
    sqY = big_pool.tile([128, NCH * YROWS], BF16)
    invY = big_pool.tile([128, YROWS], FP32)
    rnyB = big_pool.tile([128, YROWS], FP32)  # 1/|y| bcast down partitions
    dvAll = big_pool.tile([1, 2 * BS], FP32)  # smax[0:BS], -smin[BS:2BS]

    sq_pool = ctx.enter_context(tc.tile_pool(name="sq", bufs=3))
    rnx_pool = ctx.enter_context(tc.tile_pool(name="rnx", bufs=2))
    h_pool = ctx.enter_context(tc.tile_pool(name="h", bufs=2))

    psA_pool = ctx.enter_context(tc.tile_pool(name="psA", bufs=1, space="PSUM"))
    npsX = psA_pool.tile([T, BS], FP32)
    ry_pool = ctx.enter_context(tc.tile_pool(name="ry", bufs=2, space="PSUM"))
    g_pool = ctx.enter_context(tc.tile_pool(name="g", bufs=4, space="PSUM"))

    Xv = Xq[:].rearrange("p (c w) -> p c w", c=NCH)
    Yv = Yq[:].rearrange("p (c w) -> p c w", c=NCH)
    sqYv = sqY[:].rearrange("p (c w) -> p c w", c=NCH)
    xsrc = textT.ap().rearrange("(c p) w -> p c w", p=128)
    ysrc = videoT.ap().rearrange("(c p) w -> p c w", p=128)

    # --- all input DMAs issued up front; transfers pipeline back-to-back ---
    HGW = XGW // 2  # half-group: 8 samples = 616 rows per DMA / sq block
    nc.sync.dma_start(Yv, ysrc)
    for g in range(NG):
        for hh in range(2):
            w0 = g * XGW + hh * HGW
            nc.sync.dma_start(Xv[:, :, w0 : w0 + HGW], xsrc[:, :, w0 : w0 + HGW])

    # --- Y: squares, dup norm matmuls, 1/|y| broadcast ---
    for c in range(NCH):
        _sq_op(nc, SQY_PAT[c], sqYv[:, c, :], Yv[:, c, :])
    HW = YROWS // 2  # psum bank is 2KB; [128, 384] fp32 halves
    for h in range(2):
        ry = ry_pool.tile([128, HW], FP32, tag="ry", name=f"ry{h}")
        for c in range(NCH):
            nc.tensor.matmul(
                ry[:, :],
                ones[:, :128],
                sqYv[:, c, h * HW : (h + 1) * HW],
                start=(c == 0),
                stop=(c == NCH - 1),
            )
        nc.vector.reciprocal(invY[:, h * HW : (h + 1) * HW], ry[:, :])
    nc.scalar.sqrt(rnyB[:, :], invY[:, :])

    # --- X groups, software-pipelined: squares+norms+G for g, finish g-1 ---
    def emit_half(g, hh, G):
        """raw similarity matmuls + squares + sqnorm matmuls for 8 samples"""
        w0 = g * XGW + hh * HGW
        w1 = w0 + HGW
        for j in range(GRP // 2):
            b = g * GRP + hh * (GRP // 2) + j
            jj = hh * (GRP // 2) + j
            for c in range(NCH):
                nc.tensor.matmul(
                    G[:, jj * F : (jj + 1) * F],
                    Xv[:, c, b * T : (b + 1) * T],
                    Yv[:, c, b * F : (b + 1) * F],
                    start=(c == 0),
                    stop=(c == NCH - 1),
                )
        sq = sq_pool.tile([128, NCH * HGW], BF16, tag="sq", name=f"sq{g}_{hh}")
        sqv = sq[:].rearrange("p (c w) -> p c w", c=NCH)
        pat = SQX_PATS[(2 * g + hh) % len(SQX_PATS)]
        for c in range(NCH):
            _sq_op(nc, pat[c], sqv[:, c, :], Xv[:, c, w0:w1])
        for j in range(GRP // 2):
            b = g * GRP + hh * (GRP // 2) + j
            for c in range(NCH):
                nc.tensor.matmul(
                    npsX[:, b : b + 1],
                    sqv[:, c, j * T : (j + 1) * T],
                    ones[:, :1],
                    start=(c == 0),
                    stop=(c == NCH - 1),
                )

    def emit_front(g):
        G = g_pool.tile([T, GRP * F], FP32, tag="g", name=f"g{g}")
        emit_half(g, 0, G)
        emit_half(g, 1, G)
        return G

    def emit_finish(g, G):
        """rnx; scale G by rny slice; f-reduce; scale by rnx; C-reduce"""
        s0 = g * GRP
        invx = rnx_pool.tile([T, GRP], FP32, tag="invx", name=f"invx{g}")
        nc.vector.reciprocal(invx[:, :], npsX[:, s0 : s0 + GRP])
        rnx = rnx_pool.tile([T, GRP], FP32, tag="rnx", name=f"rnx{g}")
        nc.scalar.sqrt(rnx[:, :], invx[:, :])
        Gs = h_pool.tile([T, GRP * F], FP32, tag="gs", name=f"gs{g}")
        _g2 = nc.gpsimd if G2_ENG == "p" else nc.vector
        _g2.tensor_tensor(
            Gs[:, :], G[:, :], rnyB[:T, s0 * F : (s0 + GRP) * F], ALU.mult
        )
        Gv = Gs[:].rearrange("p (j f) -> p j f", f=F)
        gmx = h_pool.tile([T, GRP], FP32, tag="gmx", name=f"gmx{g}")
        gmn = h_pool.tile([T, GRP], FP32, tag="gmn", name=f"gmn{g}")
        nc.vector.tensor_reduce(gmx[:, :], Gv, axis=AX.X, op=ALU.max)
        # negate: gmn = -min_f(Gs); cross-lane reduce only supports max
        nc.vector.tensor_reduce(gmn[:, :], Gv, axis=AX.X, op=ALU.min, negate=True)
        hmx = h_pool.tile([T, GRP], FP32, tag="hmx", name=f"hmx{g}")
        hmn = h_pool.tile([T, GRP], FP32, tag="hmn", name=f"hmn{g}")
        _mult = nc.gpsimd if MULT_ENG == "p" else nc.vector
        _mult.tensor_tensor(hmx[:, :], gmx[:, :], rnx[:, :], ALU.mult)
        _mult.tensor_tensor(hmn[:, :], gmn[:, :], rnx[:, :], ALU.mult)
        nc.gpsimd.tensor_reduce(
            dvAll[:, s0 : s0 + GRP], hmx[:, :], axis=AX.C, op=ALU.max
        )
        nc.gpsimd.tensor_reduce(
            dvAll[:, BS + s0 : BS + s0 + GRP], hmn[:, :], axis=AX.C, op=ALU.max
        )

    prev = None
    for g in range(NG):
        G = emit_front(g)
        if prev is not None:
            emit_finish(*prev)
        prev = (g, G)
    emit_finish(*prev)

    nc.sync.dma_start(dout.ap(), dvAll[:, :])


def build():
    nc = bass.Bass("TRN2", target_bir_lowering=False, debug=False)
    textT = nc.dram_tensor("textT", [D, XROWS], X_DT, kind="ExternalInput")
    videoT = nc.dram_tensor("videoT", [D, YROWS], Y_DT, kind="ExternalInput")
    dout = nc.dram_tensor("dout", [1, 2 * BS], FP32, kind="ExternalOutput")
    with tile.TileContext(nc) as tc:
        with ExitStack() as ctx:
            _build_body(ctx, tc, textT, videoT, dout)
    _split_sync_waits(nc)
    return nc


_nc_cache = None


def _get_nc():
    global _nc_cache
    if _nc_cache is None:
        _nc_cache = build()
    return _nc_cache


def _np_dt(dt):
    import ml_dtypes

    if dt == FP8:
        return np.dtype(ml_dtypes.float8_e4m3)
    return np.dtype(ml_dtypes.bfloat16)


def prep_core_inputs(text: np.ndarray, video: np.ndarray, core: int) -> dict:
    """low-precision cast + D-major transpose of one core's shard."""
    xs = text[core * BS : (core + 1) * BS].astype(_np_dt(X_DT)).reshape(XROWS, D).T
    ys = video[core * BS : (core + 1) * BS].astype(_np_dt(Y_DT)).reshape(YROWS, D).T
    return {
        "textT": np.ascontiguousarray(xs),
        "videoT": np.ascontiguousarray(ys),
    }


def run_device(text: np.ndarray, video: np.ndarray, trace: bool = False):
    """Run the SPMD kernel on 8 cores; returns (smax[B], smin[B], results)."""
    nc = _get_nc()
    in_maps = [prep_core_inputs(text, video, i) for i in range(NCORES)]
    res = run_bass_kernel_spmd(nc, in_maps, list(range(NCORES)), trace=trace)
    douts = [np.asarray(res.results[i]["dout"]) for i in range(NCORES)]
    smax = np.concatenate([d[0, :BS] for d in douts])
    smin = np.concatenate([-d[0, BS:] for d in douts])
    return smax, smin, res


def kernel(Prob_text: np.ndarray, Prob_video: np.ndarray) -> np.ndarray:
    text = np.ascontiguousarray(np.asarray(Prob_text, dtype=np.float32))
    video = np.ascontiguousarray(np.asarray(Prob_video, dtype=np.float32))
    smax, smin, _ = run_device(text, video)
    dmin = 1.0 - smax.astype(np.float64)
    dmax = 1.0 - smin.astype(np.float64)
    mn = dmin.min()
    mx = dmax.max()
    dis = np.broadcast_to(dmax[:, None], (B, B)).copy()
    np.fill_diagonal(dis, dmin)
    return ((dis - mn) / (mx - mn)).astype(np.float32)


# revision 28
# speedup vs baseline: 1.2099x; 1.0101x over previous
"""Trainium2 Bass kernel for nn_Distance_Module (retrieval_knn).

Math: out[i,j] = (dmax[i]-mn)/(mx-mn) off-diagonal, (dmin[i]-mn)/(mx-mn)
on the diagonal, where per sample i:
  s[t,f] = <text[i,t]/|..|, video[i,f]/|..|>, dmin[i] = 1-max s, dmax[i] = 1-min s,
mn = min_i dmin[i], mx = max_i dmax[i].

Device kernel (SPMD x8, batch-sharded, no collectives): per core 64 samples.
Host uploads each core's shard pre-cast to bf16 and pre-transposed to
D-major ([512, rows]); the device reads it with large contiguous DMA
descriptors (memory-roofline bound), computes row sq-norms with one bf16
squares pass + per-sample ones-matmuls (PSUM column accumulation), raw
similarity matrices with per-sample bf16 matmuls, folds the 1/(|x||y|)
normalization in as a rank-1 outer-product tile, and reduces min/max per
sample. Host: gather 8x[64,2] vectors, global min/max, build [512,512].
"""

from contextlib import ExitStack

import numpy as np

import concourse.bass as bass
import concourse.tile as tile
from concourse import masks, mybir
from concourse.bass_utils import run_bass_kernel_spmd
from concourse.vector_clock import ScopedClock

# The walrus in this toolchain only allows ONE sync-wait per instruction;
# TileContext's tail drain attaches one wait per outstanding semaphore and
# fails codegen. Split them across consecutive drains / NoOps.
_MAX_CTRL_WAITS = 1


def _split_drain_and_barrier(self, tick_clock, wait_clock):
    nc = self.nc
    drain_inst = nc.sync.drain()
    wait_clock.add_sem_waits(
        drain_inst.ins, ScopedClock({None: tick_clock.global_clock})
    )
    si = drain_inst.ins.sync_info
    waits = list(si.on_wait or []) if si else []
    if len(waits) > _MAX_CTRL_WAITS:
        si.on_wait = waits[:_MAX_CTRL_WAITS]
        for i in range(_MAX_CTRL_WAITS, len(waits), _MAX_CTRL_WAITS):
            extra = nc.sync.drain()
            esi = extra.ins.sync_info
            chunk = waits[i : i + _MAX_CTRL_WAITS]
            if esi is None:
                extra.ins.sync_info = mybir.SyncInfo(on_wait=chunk, on_update=[])
            else:
                esi.on_wait = chunk
    nc.all_engine_barrier()
    assert self.sems is not None
    popped = nc._tile_sem_poison_stack.pop()
    assert popped is self._sem_poison
    nc.clear_and_free_semaphores(list(self.sems.allocated().values()))
    # No trailing all-engine barrier: the clears are the last instructions,
    # NEFF completion already waits for every engine queue to retire, and
    # this is the outermost (only) TileContext so nothing follows.


tile.TileContext._drain_and_barrier = _split_drain_and_barrier


def _split_sync_waits(nc, max_waits=_MAX_CTRL_WAITS):
    """Hoist extra sync-waits onto same-engine NoOps inserted just before
    the offending instruction."""
    f = nc.m.functions[0]
    for blk in f.blocks:
        out = []
        for inst in blk.instructions:
            si = getattr(inst, "sync_info", None)
            waits = list(si.on_wait) if (si and si.on_wait) else []
            if len(waits) > max_waits:
                for i in range(0, len(waits) - max_waits, max_waits):
                    nop = mybir.InstNoOp(
                        name=nc.get_next_instruction_name(), ins=[], outs=[]
                    )
                    nop.engine = inst.engine
                    nop.sync_info = mybir.SyncInfo(
                        on_wait=waits[i : i + max_waits], on_update=[]
                    )
                    nc.register_instruction(nop)
                    out.append(nop)
                si.on_wait = waits[len(waits) - max_waits :]
            out.append(inst)
        blk.instructions[:] = out


B, T, F, D = 512, 77, 12, 512
NCORES = 8
BS = B // NCORES  # 64 samples per core
XROWS = BS * T  # 4928
YROWS = BS * F  # 768
NCH = D // 128  # 4 contraction chunks
GRP = 16  # samples per finish group
NG = BS // GRP  # 4 groups
XGW = GRP * T  # 1232 x-rows per group

FP32 = mybir.dt.float32
BF16 = mybir.dt.bfloat16
FP8 = mybir.dt.float8e4
ALU = mybir.AluOpType
AX = mybir.AxisListType
ACTF = mybir.ActivationFunctionType

# Input dtypes for similarity operands: BF16 or FP8 (e4m3).
X_DT = FP8
Y_DT = BF16

# Engine split tables: "d"=DVE tensor_tensor, "a"=ACT Square, "p"=Pool TT.
SQY_PAT = "dada"  # y squares, per chunk (768 cols each)
SQX_PATS = ["dapd", "adpa", "dapd", "adpa", "dapd", "adpa", "aapd", "adpa"]  # per half-group, cycled
G2_ENG = "d"  # must not be "p": gpsimd cannot read PSUM  # G*rny scale engine (p=Pool, d=DVE)
MULT_ENG = "p"  # hmx/hmn scale engine


def _sq_op(nc, eng, out_ap, in_ap):
    if eng == "a":
        nc.scalar.activation(out_ap, in_ap, ACTF.Square, 0.0, 1.0)
    elif eng == "d":
        nc.vector.tensor_tensor(out_ap, in_ap, in_ap, ALU.mult)
    else:
        nc.gpsimd.tensor_tensor(out_ap, in_ap, in_ap, ALU.mult)


def _sq_grouped(nc, pat, sqv, Xv, w0, w1):
    """Emit squares ops, merging same-engine chunks into strided-AP ops."""
    byeng = {}
    for c in range(len(pat)):
        byeng.setdefault(pat[c], []).append(c)
    for eng, cs in byeng.items():
        runs = [(c, 1, 1) for c in cs]  # merging disabled: device mismatch
        if False:
            pass
        for (c0, st, n) in runs:
            if n == 1:
                _sq_op(nc, eng, sqv[:, c0, :], Xv[:, c0, w0:w1])
            else:
                _sq_op(nc, eng, sqv[:, c0::st, :][:, :n, :],
                       Xv[:, c0::st, w0:w1][:, :n, :])


def _build_body(ctx: ExitStack, tc: "tile.TileContext", textT, videoT, dout):
    nc = tc.nc

    const_pool = ctx.enter_context(tc.tile_pool(name="const", bufs=1))
    ones = const_pool.tile([128, 128], BF16)
    nc.gpsimd.memset(ones[:], 1.0)

    # Persistent D-major shards: [128, c, rows] (partition = d%128).
    big_pool = ctx.enter_context(tc.tile_pool(name="big", bufs=1))
    Xq = big_pool.tile([128, NCH * XROWS], X_DT)
    Yq = big_pool.tile([128, NCH * YROWS], Y_DT)
    sqY = big_pool.tile([128, NCH * YROWS], BF16)
    invY = big_pool.tile([128, YROWS], FP32)
    rnyB = big_pool.tile([128, YROWS], FP32)  # 1/|y| bcast down partitions
    dvAll = big_pool.tile([1, 2 * BS], FP32)  # smax[0:BS], -smin[BS:2BS]

    sq_pool = ctx.enter_context(tc.tile_pool(name="sq", bufs=3))
    rnx_pool = ctx.enter_context(tc.tile_pool(name="rnx", bufs=2))
    h_pool = ctx.enter_context(tc.tile_pool(name="h", bufs=2))

    psA_pool = ctx.enter_context(tc.tile_pool(name="psA", bufs=1, space="PSUM"))
    npsX = psA_pool.tile([T, BS], FP32)
    ry_pool = ctx.enter_context(tc.tile_pool(name="ry", bufs=2, space="PSUM"))
    g_pool = ctx.enter_context(tc.tile_pool(name="g", bufs=4, space="PSUM"))

    Xv = Xq[:].rearrange("p (c w) -> p c w", c=NCH)
    Yv = Yq[:].rearrange("p (c w) -> p c w", c=NCH)
    sqYv = sqY[:].rearrange("p (c w) -> p c w", c=NCH)
    xsrc = textT.ap().rearrange("(c p) w -> p c w", p=128)
    ysrc = videoT.ap().rearrange("(c p) w -> p c w", p=128)

    # --- all input DMAs issued up front; transfers pipeline back-to-back ---
    HGW = XGW // 2  # half-group: 8 samples = 616 rows per DMA / sq block
    nc.sync.dma_start(Yv, ysrc)
    for g in range(NG):
        for hh in range(2):
            w0 = g * XGW + hh * HGW
            nc.sync.dma_start(Xv[:, :, w0 : w0 + HGW], xsrc[:, :, w0 : w0 + HGW])

    # --- Y: squares, dup norm matmuls, 1/|y| broadcast ---
    _sq_grouped(nc, SQY_PAT, sqYv, Yv, 0, YROWS)
    HW = YROWS // 2  # psum bank is 2KB; [128, 384] fp32 halves
    for h in range(2):
        ry = ry_pool.tile([128, HW], FP32, tag="ry", name=f"ry{h}")
        for c in range(NCH):
            nc.tensor.matmul(
                ry[:, :],
                ones[:, :128],
                sqYv[:, c, h * HW : (h + 1) * HW],
                start=(c == 0),
                stop=(c == NCH - 1),
            )
        nc.vector.reciprocal(invY[:, h * HW : (h + 1) * HW], ry[:, :])
    nc.scalar.sqrt(rnyB[:, :], invY[:, :])

    # --- X groups, software-pipelined: squares+norms+G for g, finish g-1 ---
    def emit_half(g, hh, G):
        """raw similarity matmuls + squares + sqnorm matmuls for 8 samples"""
        w0 = g * XGW + hh * HGW
        w1 = w0 + HGW
        for j in range(GRP // 2):
            b = g * GRP + hh * (GRP // 2) + j
            jj = hh * (GRP // 2) + j
            for c in range(NCH):
                nc.tensor.matmul(
                    G[:, jj * F : (jj + 1) * F],
                    Xv[:, c, b * T : (b + 1) * T],
                    Yv[:, c, b * F : (b + 1) * F],
                    start=(c == 0),
                    stop=(c == NCH - 1),
                )
        sq = sq_pool.tile([128, NCH * HGW], BF16, tag="sq", name=f"sq{g}_{hh}")
        sqv = sq[:].rearrange("p (c w) -> p c w", c=NCH)
        pat = SQX_PATS[(2 * g + hh) % len(SQX_PATS)]
        _sq_grouped(nc, pat, sqv, Xv, w0, w1)
        for j in range(GRP // 2):
            b = g * GRP + hh * (GRP // 2) + j
            for c in range(NCH):
                nc.tensor.matmul(
                    npsX[:, b : b + 1],
                    sqv[:, c, j * T : (j + 1) * T],
                    ones[:, :1],
                    start=(c == 0),
                    stop=(c == NCH - 1),
                )

    def emit_front(g):
        G = g_pool.tile([T, GRP * F], FP32, tag="g", name=f"g{g}")
        emit_half(g, 0, G)
        emit_half(g, 1, G)
        return (G,)

    def emit_finish(g, G):
        """scale G by rny; f-reduce; rnx from sqnorms; scale; C-reduce"""
        s0 = g * GRP
        invx = rnx_pool.tile([T, GRP], FP32, tag="invx", name=f"invx{g}")
        nc.vector.reciprocal(invx[:, :], npsX[:, s0 : s0 + GRP])
        rnx = rnx_pool.tile([T, GRP], FP32, tag="rnx", name=f"rnx{g}")
        nc.scalar.sqrt(rnx[:, :], invx[:, :])
        Gs = h_pool.tile([T, GRP * F], FP32, tag="gs", name=f"gs{g}")
        _g2 = nc.gpsimd if G2_ENG == "p" else nc.vector
        _g2.tensor_tensor(
            Gs[:, :], G[:, :], rnyB[:T, s0 * F : (s0 + GRP) * F], ALU.mult
        )
        Gv = Gs[:].rearrange("p (j f) -> p j f", f=F)
        gmx = h_pool.tile([T, GRP], FP32, tag="gmx", name=f"gmx{g}")
        gmn = h_pool.tile([T, GRP], FP32, tag="gmn", name=f"gmn{g}")
        nc.vector.tensor_reduce(gmx[:, :], Gv, axis=AX.X, op=ALU.max)
        # negate: gmn = -min_f(Gs); cross-lane reduce only supports max
        nc.vector.tensor_reduce(gmn[:, :], Gv, axis=AX.X, op=ALU.min, negate=True)
        hmx = h_pool.tile([T, GRP], FP32, tag="hmx", name=f"hmx{g}")
        hmn = h_pool.tile([T, GRP], FP32, tag="hmn", name=f"hmn{g}")
        _mult = nc.gpsimd if MULT_ENG == "p" else nc.vector
        _mult.tensor_tensor(hmx[:, :], gmx[:, :], rnx[:, :], ALU.mult)
        _mult.tensor_tensor(hmn[:, :], gmn[:, :], rnx[:, :], ALU.mult)
        nc.gpsimd.tensor_reduce(
            dvAll[:, s0 : s0 + GRP], hmx[:, :], axis=AX.C, op=ALU.max
        )
        nc.gpsimd.tensor_reduce(
            dvAll[:, BS + s0 : BS + s0 + GRP], hmn[:, :], axis=AX.C, op=ALU.max
        )

    prev = None
    for g in range(NG):
        state = emit_front(g)
        if prev is not None:
            emit_finish(*prev)
        prev = (g, *state)
    emit_finish(*prev)

    nc.sync.dma_start(dout.ap(), dvAll[:, :])


def build():
    nc = bass.Bass("TRN2", target_bir_lowering=False, debug=False)
    textT = nc.dram_tensor("textT", [D, XROWS], X_DT, kind="ExternalInput")
    videoT = nc.dram_tensor("videoT", [D, YROWS], Y_DT, kind="ExternalInput")
    dout = nc.dram_tensor("dout", [1, 2 * BS], FP32, kind="ExternalOutput")
    with tile.TileContext(nc) as tc:
        with ExitStack() as ctx:
            _build_body(ctx, tc, textT, videoT, dout)
    _split_sync_waits(nc)
    return nc


_nc_cache = None


def _get_nc():
    global _nc_cache
    if _nc_cache is None:
        _nc_cache = build()
    return _nc_cache


def _np_dt(dt):
    import ml_dtypes

    if dt == FP8:
        return np.dtype(ml_dtypes.float8_e4m3)
    return np.dtype(ml_dtypes.bfloat16)


def prep_core_inputs(text: np.ndarray, video: np.ndarray, core: int) -> dict:
    """low-precision cast + D-major transpose of one core's shard."""
    xs = text[core * BS : (core + 1) * BS].astype(_np_dt(X_DT)).reshape(XROWS, D).T
    ys = video[core * BS : (core + 1) * BS].astype(_np_dt(Y_DT)).reshape(YROWS, D).T
    return {
        "textT": np.ascontiguousarray(xs),
        "videoT": np.ascontiguousarray(ys),
    }


def run_device(text: np.ndarray, video: np.ndarray, trace: bool = False):
    """Run the SPMD kernel on 8 cores; returns (smax[B], smin[B], results)."""
    nc = _get_nc()
    in_maps = [prep_core_inputs(text, video, i) for i in range(NCORES)]
    res = run_bass_kernel_spmd(nc, in_maps, list(range(NCORES)), trace=trace)
    douts = [np.asarray(res.results[i]["dout"]) for i in range(NCORES)]
    smax = np.concatenate([d[0, :BS] for d in douts])
    smin = np.concatenate([-d[0, BS:] for d in douts])
    return smax, smin, res


def kernel(Prob_text: np.ndarray, Prob_video: np.ndarray) -> np.ndarray:
    text = np.ascontiguousarray(np.asarray(Prob_text, dtype=np.float32))
    video = np.ascontiguousarray(np.asarray(Prob_video, dtype=np.float32))
    smax, smin, _ = run_device(text, video)
    dmin = 1.0 - smax.astype(np.float64)
    dmax = 1.0 - smin.astype(np.float64)
    mn = dmin.min()
    mx = dmax.max()
    dis = np.broadcast_to(dmax[:, None], (B, B)).copy()
    np.fill_diagonal(dis, dmin)
    return ((dis - mn) / (mx - mn)).astype(np.float32)
